# revision 31
# baseline (speedup 1.0000x reference)
"""Trainium2 Bass kernel for nn_Attention (dense transformer attention block).

Reference computation (shapes fixed):
  x [2, 256, 48, 48] -> RMSNorm over channels -> 1x1 conv to qkv (8 heads, 64 dhead)
  -> prepend 4 learnable mem kv tokens -> softmax attention -> 1x1 conv out [2, 256, 48, 48]

Sharding (v4, head-sharded, collective-free): 8 cores = 2 batches x 4
head-pairs. Core c handles batch c//4 and heads (2g, 2g+1), g = c%4. Each core
runs its heads' attention over ALL 2304 queries (512-wide chunks keep the PE
at peak column rate) and emits its PARTIAL out-projection [256, 2304] in f32,
DMA'd straight from PSUM. The host sums the 4 partials per batch -- no
on-device collective at all.

Key structure:
  - x is L2-normalized ONCE (xn = x * sigma, sigma=16/l2 per position, bf16)
    before the qkv projections, so q, k, v all come out normalized: no per-key
    exp bias, no sigma column machinery; the attnv denominator column is
    exactly 1.0.
  - attnv lhsT staging per key tile: [keys, 2 heads, 128], col 0 = 1.0 (den),
    cols 64:128 = v^T; acc row 0 = denominator, rows 64:128 = attention out.
  - exp split across ACT (table exp) and DVE (Schraudolph bf16 bit-trick).
  - PE pipeline: sim(jt+1) emitted before attnv(jt); sim head pairs run
    concurrently on disjoint PE row groups (K=64 at base partitions 0/64).
  - engine placement: all input DMAs on the sync queue; gpsimd = xn muls +
    staging zero-memsets + partition broadcasts; ACT = squares/sqrts/qb cast +
    exp share; DVE = kb/v staging, oT normalize, recips + exp share.
"""
import math

import numpy as np

import concourse.mybir as mybir
import concourse.tile as tile
from concourse import bacc
from concourse.bass_utils import run_bass_kernel_spmd


F32 = mybir.dt.float32
F32R = mybir.dt.float32r
BF16 = mybir.dt.bfloat16
U16 = mybir.dt.uint16
EXP = mybir.ActivationFunctionType.Exp
SQRT = mybir.ActivationFunctionType.Sqrt
SQUARE = mybir.ActivationFunctionType.Square
COPY = mybir.ActivationFunctionType.Copy
MULT = mybir.AluOpType.mult
ADD = mybir.AluOpType.add

DIM = 256
HEADS = 8
DHEAD = 64
MEM = 4
HID = 512
N = 48 * 48          # 2304 image positions
NJT = 18             # image key tiles of 128

CHUNKS = [(0, 512), (512, 512), (1024, 512), (1536, 512), (2048, 256)]
NCH = len(CHUNKS)
DVE_JTS = frozenset({2, 4, 7, 9, 11, 14, 16, 17})  # interleaved with ACT

A_SCH = 128.0 / math.log(2.0)        # bf16 Schraudolph slope
B_SCH = 127.0 * 128.0 - 5.5          # minimax-centered magic constant


def _jt_slice(jt):
    """key tile jt -> (chunk index, col offset within chunk)."""
    pos0 = jt * 128
    for ci, (c0, cw) in enumerate(CHUNKS):
        if c0 <= pos0 < c0 + cw:
            return ci, pos0 - c0
    raise AssertionError(jt)


def build():
    nc = bacc.Bacc("TRN2", target_bir_lowering=False, debug=False,
                   enable_asserts=True, num_devices=8)
    x_d = nc.dram_tensor("x", [DIM, N], F32, kind="ExternalInput").ap()
    wqkv_d = nc.dram_tensor("wqkv", [DIM, 384], F32, kind="ExternalInput").ap()
    memk_d = nc.dram_tensor("memk", [128, MEM], F32, kind="ExternalInput").ap()
    memvst_d = nc.dram_tensor("memvst", [MEM, 2, 128], F32,
                              kind="ExternalInput").ap()
    woutT_d = nc.dram_tensor("woutT", [2, DHEAD, DIM], F32,
                             kind="ExternalInput").ap()
    out_d = nc.dram_tensor("out", [2, 128, N], BF16,
                           kind="ExternalOutput").ap()

    with tile.TileContext(nc) as tc:
        with (
            tc.tile_pool(name="consts", bufs=1) as consts,
            tc.tile_pool(name="big", bufs=1) as big,
            tc.tile_pool(name="io", bufs=1) as io,
            tc.tile_pool(name="wk", bufs=2) as wk,
            tc.tile_pool(name="ps_s", bufs=2, space="PSUM") as ps_s,
            tc.tile_pool(name="ps_a", bufs=2, space="PSUM") as ps_a,
        ):
            # ---------------- input DMAs (sync queue, priority order) ------
            xs = [[None] * NCH, [None] * NCH]
            for ci, (c0, cw) in enumerate(CHUNKS):
                for kt in range(2):
                    xs[kt][ci] = big.tile([128, 512], F32, tag=f"x{kt}_{ci}",
                                          name=f"x{kt}_{ci}")
            wq_f = io.tile([128, 2, 384], F32, tag="wq_f")
            memk_f = io.tile([128, MEM], F32, tag="memk_f")
            memvst_f = io.tile([MEM, 2, 128], F32, tag="memv_f")
            woutA_f = io.tile([128, DIM], F32, tag="woutA_f")
            woutB_f = io.tile([128, DIM], F32, tag="woutB_f")

            # spread x across the 3 DMA-capable queues so late chunks land
            # early; scalar carries the late-needed chunks (issue cost only).
            nc.sync.dma_start(out=xs[0][0][:, :], in_=x_d[0:128, 0:512])
            nc.gpsimd.dma_start(out=xs[1][0][:, :], in_=x_d[128:256, 0:512])
            nc.sync.dma_start(out=wq_f[:, 0, :], in_=wqkv_d[0:128, :])
            nc.gpsimd.dma_start(out=wq_f[:, 1, :], in_=wqkv_d[128:256, :])
            nc.scalar.dma_start(out=xs[0][2][:, :], in_=x_d[0:128, 1024:1536])
            nc.scalar.dma_start(out=xs[1][2][:, :], in_=x_d[128:256, 1024:1536])
            nc.sync.dma_start(out=xs[0][1][:, :], in_=x_d[0:128, 512:1024])
            nc.gpsimd.dma_start(out=xs[1][1][:, :], in_=x_d[128:256, 512:1024])
            nc.scalar.dma_start(out=xs[0][4][:, 0:256], in_=x_d[0:128, 2048:2304])
            nc.scalar.dma_start(out=xs[1][4][:, 0:256], in_=x_d[128:256, 2048:2304])
            nc.sync.dma_start(out=xs[0][3][:, :], in_=x_d[0:128, 1536:2048])
            nc.gpsimd.dma_start(out=xs[1][3][:, :], in_=x_d[128:256, 1536:2048])
            nc.sync.dma_start(out=memk_f[:, :], in_=memk_d)
            nc.sync.dma_start(out=memvst_f[:, :, :], in_=memvst_d)
            nc.sync.dma_start(out=woutA_f[64:128, :], in_=woutT_d[0, :, :])
            nc.gpsimd.dma_start(out=woutB_f[64:128, :], in_=woutT_d[1, :, :])

            # ---------------- constants / staging init --------------------
            ones_f = consts.tile([128, 1], F32)
            nc.vector.memset(ones_f[:, :], 1.0)
            ones_r = consts.tile([128, 128], F32R)
            nc.vector.tensor_copy(ones_r[:, :],
                                  ones_f[:, :].to_broadcast((128, 128)))



            # attnv lhsT staging: [keys, 2 heads, 128]
            # col 0 = 1.0 (denominator), 1:64 zeros, 64:128 = v^T
            vstag = [big.tile([128, 2, 128], BF16, tag=f"vst{jt}",
                              name=f"vst{jt}")
                     for jt in range(NJT + 1)]
            for jt in range(NJT):
                nc.vector.memset(vstag[jt][:, :, 0:1], 1.0)

            # weight conversion: bf16 qkv weights
            wq = consts.tile([128, 2, 384], BF16, tag="wq")
            nc.vector.tensor_copy(wq[:, :, :], wq_f[:, :, :])

            kb = [None] * NCH
            qb = [None] * NCH

            # ---------------- prep: per-chunk qkv --------------------------
            def prep_chunk(ci):
                c0, cw = CHUNKS[ci]
                njs = cw // 128
                xsq = wk.tile([128, 2, 512], F32R, tag="xsq", name=f"xsq{ci}")
                nc.scalar.activation(xsq[:, 0, 0:cw],
                                     xs[0][ci][:, 0:cw], SQUARE)
                nc.vector.tensor_mul(xsq[:, 1, 0:cw], xs[1][ci][:, 0:cw],
                                     xs[1][ci][:, 0:cw])
                ssq = ps_a.tile([128, 2, 512], F32, tag="a", name=f"ssq{ci}")
                for kt in range(2):
                    nc.tensor.matmul(ssq[:, 0, 0:cw], ones_r[:, :],
                                     xsq[:, kt, 0:cw],
                                     start=(kt == 0), stop=(kt == 1))
                # sigma rows = 16/l2 (sqrt then fast reciprocal)
                srow = wk.tile([128, 512], F32, tag="sr", name=f"sr{ci}")
                nc.scalar.activation(srow[:, 0:cw], ssq[:, 0, 0:cw], SQRT,
                                     scale=1.0 / 256.0)
                nc.vector.reciprocal_approx_fast(srow[:, 0:cw], srow[:, 0:cw])
                # xn = x * sigma (normalized x, bf16 for the matmuls)
                xn = wk.tile([128, 2, 512], BF16, tag="xn", name=f"xn{ci}")
                nc.vector.tensor_mul(xn[:, 0, 0:cw], xs[0][ci][:, 0:cw],
                                     srow[:, 0:cw])
                nc.gpsimd.tensor_mul(xn[:, 1, 0:cw], xs[1][ci][:, 0:cw],
                                     srow[:, 0:cw])
                # q, k projections
                qk = ps_s.tile([128, 2, 512], F32, tag="s", name=f"qk_{ci}")
                for m in range(2):  # 0 = q, 1 = k
                    for kt in range(2):
                        nc.tensor.matmul(
                            qk[:, m, 0:cw],
                            wq[:, kt, m * 128:m * 128 + 128],
                            xn[:, kt, 0:cw],
                            start=(kt == 0), stop=(kt == 1))
                qb[ci] = big.tile([128, 512], BF16, tag=f"q{ci}",
                                  name=f"qb{ci}")
                kb[ci] = big.tile([128, 512], BF16, tag=f"k{ci}",
                                  name=f"kb{ci}")
                nc.scalar.activation(qb[ci][:, 0:cw], qk[:, 0, 0:cw], COPY)
                nc.vector.tensor_copy(kb[ci][:, 0:cw], qk[:, 1, 0:cw])
                # v, directly transposed: [pos, 2 heads, 64]
                vps = ps_a.tile([128, 2, 4, 2, 64], F32, tag="a",
                                name=f"vps{ci}")
                for js in range(njs):
                    jt = ci * 4 + js
                    nc.gpsimd.memset(vstag[jt][:, :, 1:64], 0.0)
                    for kt in range(2):
                        nc.tensor.matmul(
                            vps[:, 0, js, :, :],
                            xn[:, kt, js * 128:js * 128 + 128],
                            wq[:, kt, 256:384],
                            start=(kt == 0), stop=(kt == 1))
                    nc.vector.tensor_copy(vstag[jt][:, :, 64:128],
                                          vps[:, 0, js, :, :])

            for ci in range(NCH):
                prep_chunk(ci)
            # late consts (off the critical path at start)
            kmem = consts.tile([128, MEM], BF16, tag="kmem")
            nc.vector.tensor_copy(kmem[:, :], memk_f[:, :])
            nc.vector.tensor_copy(vstag[NJT][0:MEM, :, :], memvst_f[:, :, :])
            woutA = consts.tile([128, DIM], BF16, tag="woutA")
            woutB = consts.tile([128, DIM], BF16, tag="woutB")
            nc.vector.tensor_copy(woutA[64:128, :], woutA_f[64:128, :])
            nc.vector.tensor_copy(woutB[64:128, :], woutB_f[64:128, :])
            wouts = [woutA, woutB]

            # ---------------- attention chunks ------------------------------
            accs_by_ci = [None] * NCH
            fin = {}

            def emit_sim(ci, jt):
                c0, cw = CHUNKS[ci]
                s_ps = ps_s.tile([128, 2, 512], F32, tag="s",
                                 name=f"s_{ci}_{jt}")
                if jt < NJT:
                    km = 128
                    kc, off = _jt_slice(jt)
                    klhs = [kb[kc][64 * h:64 * h + 64, off:off + 128]
                            for h in range(2)]
                else:
                    km = MEM
                    klhs = [kmem[64 * h:64 * h + 64, :] for h in range(2)]
                for h in range(2):
                    nc.tensor.matmul(
                        s_ps[0:km, h, 0:cw],
                        klhs[h],
                        qb[ci][64 * h:64 * h + 64, 0:cw],
                        start=True, stop=True)
                return s_ps, km

            def emit_exp(ci, jt, s_ps, km):
                c0, cw = CHUNKS[ci]
                if jt in DVE_JTS:
                    P = wk.tile([128, 2, 512], U16, tag="Pd",
                                name=f"Pd_{ci}_{jt}")
                    nc.vector.tensor_scalar(
                        out=P[:, :, 0:cw], in0=s_ps[:, :, 0:cw],
                        scalar1=A_SCH, scalar2=B_SCH,
                        op0=MULT, op1=ADD)
                    return P, True
                P = wk.tile([128, 2, 512], BF16, tag="Pa", name=f"Pa_{ci}_{jt}")
                nc.scalar.activation(P[0:km, :, 0:cw], s_ps[0:km, :, 0:cw],
                                     EXP)
                return P, False

            def emit_attnv(ci, jt, P, km, cast):
                c0, cw = CHUNKS[ci]
                acc = accs_by_ci[ci]
                for h in range(2):
                    rhs = P[0:km, h, 0:cw]
                    if cast:
                        rhs = rhs.bitcast(BF16)
                    nc.tensor.matmul(
                        acc[:, h, 0:cw],
                        vstag[jt][0:km, h, :],
                        rhs,
                        start=(jt == 0), stop=(jt == NJT),
                        skip_group_check=True)

            def emit_fin_a(ci):
                """den reciprocal + partition broadcast."""
                c0, cw = CHUNKS[ci]
                acc = accs_by_ci[ci]
                rec = wk.tile([128, 2, 512], F32, tag="rec", name=f"rec{ci}")
                rb = wk.tile([128, 2, 512], F32, tag="rb", name=f"rb{ci}")
                nc.vector.reciprocal_approx_fast(rec[0:1, :, 0:cw],
                                                 acc[0:1, :, 0:cw])
                nc.gpsimd.partition_broadcast(rb[:, :, 0:cw],
                                              rec[0:1, :, 0:cw])
                fin[ci] = rb

            def emit_fin_b(ci):
                """normalize oT (DVE)."""
                c0, cw = CHUNKS[ci]
                acc = accs_by_ci[ci]
                rb = fin.pop(ci)
                oT = wk.tile([128, 2, 512], BF16, tag="oT", name=f"oT_{ci}")
                nc.vector.tensor_mul(oT[64:128, :, 0:cw],
                                     acc[64:128, :, 0:cw],
                                     rb[64:128, :, 0:cw])
                fin[ci] = oT

            def emit_fin_c(ci):
                """partial out-projection (PE)."""
                c0, cw = CHUNKS[ci]
                oT = fin.pop(ci)
                op = ps_s.tile([128, 2, 512], F32, tag="s", name=f"op_{ci}")
                for mt in range(2):
                    for h in range(2):
                        nc.tensor.matmul(
                            op[:, mt, 0:cw],
                            wouts[h][64:128, mt * 128:mt * 128 + 128],
                            oT[64:128, h, 0:cw],
                            start=(h == 0), stop=(h == 1))
                fin[ci] = op

            def emit_fin_d(ci):
                """osb cast (ACT) + output DMA (sync)."""
                c0, cw = CHUNKS[ci]
                op = fin.pop(ci)
                osb = wk.tile([128, 2, 512], BF16, tag="osb", name=f"osb{ci}")
                nc.scalar.activation(osb[:, :, 0:cw], op[:, :, 0:cw], COPY)
                for mt in range(2):
                    nc.sync.dma_start(out=out_d[mt, :, c0:c0 + cw],
                                      in_=osb[:, mt, 0:cw])

            def hooks(ci, jt):
                if ci == 0:
                    return
                if jt == 3:
                    emit_fin_a(ci - 1)
                elif jt == 5:
                    emit_fin_b(ci - 1)
                elif jt == 8:
                    emit_fin_c(ci - 1)
                elif jt == 10:
                    emit_fin_d(ci - 1)

            for ci, (c0, cw) in enumerate(CHUNKS):
                accs_by_ci[ci] = ps_a.tile([128, 2, 512], F32, tag="a",
                                           name=f"acc_{ci}")
                pend = None
                for jt in range(NJT + 1):
                    s_ps, km = emit_sim(ci, jt)
                    if pend is not None:
                        emit_attnv(ci, *pend)
                    hooks(ci, jt)
                    P, cast = emit_exp(ci, jt, s_ps, km)
                    pend = (jt, P, km, cast)
                emit_attnv(ci, *pend)
            emit_fin_a(NCH - 1)
            emit_fin_b(NCH - 1)
            emit_fin_c(NCH - 1)
            emit_fin_d(NCH - 1)
    nc.compile()
    return nc


_NC = None
_last_in_maps = None


def _get_nc():
    global _NC
    if _NC is None:
        _NC = build()
    return _NC


def make_in_maps(x, gamma, mem_kv, w_qkv, w_out):
    x = np.asarray(x, np.float32)
    gamma = np.asarray(gamma, np.float32).reshape(DIM)
    mem_kv = np.asarray(mem_kv, np.float32)
    w_qkv = np.asarray(w_qkv, np.float32)
    w_out = np.asarray(w_out, np.float32)

    g1 = 1.0 + gamma  # [256]
    scale = DHEAD ** -0.5
    in_maps = []
    for core in range(8):
        b, g = core // 4, core % 4
        hA, hB = 2 * g, 2 * g + 1
        blocks = []
        for t in range(3):  # q, k, v
            for h in (hA, hB):
                wblk = w_qkv[t * HID + h * DHEAD: t * HID + (h + 1) * DHEAD, :]
                if t == 0:
                    wblk = wblk * scale
                blocks.append(wblk.T)  # [256, 64]
        wqkvT = np.concatenate(blocks, axis=1) * g1[:, None]  # [256, 384]
        memk = np.concatenate(
            [mem_kv[0, hA].T, mem_kv[0, hB].T], axis=0)  # [128, 4]
        # mem staging: [4, 2, 128] = [1.0 (den) | zeros | v (64:128)]
        memvst = np.zeros((MEM, 2, 128), np.float32)
        memvst[:, :, 0] = 1.0
        memvst[:, 0, 64:128] = mem_kv[1, hA]
        memvst[:, 1, 64:128] = mem_kv[1, hB]
        # wout rows: head A weights at partitions 64:128, head B at 0:64
        woutT = np.stack(
            [w_out[:, hA * DHEAD:(hA + 1) * DHEAD].T,
             w_out[:, hB * DHEAD:(hB + 1) * DHEAD].T], axis=0)  # [2, 64, 256]
        in_maps.append({
            "x": np.ascontiguousarray(x[b].reshape(DIM, N)),
            "wqkv": np.ascontiguousarray(wqkvT),
            "memk": np.ascontiguousarray(memk),
            "memvst": np.ascontiguousarray(memvst),
            "woutT": np.ascontiguousarray(woutT),
        })
    return in_maps


def kernel(x, gamma, mem_kv, w_qkv, w_out):
    global _last_in_maps
    in_maps = make_in_maps(x, gamma, mem_kv, w_qkv, w_out)
    _last_in_maps = in_maps
    nc = _get_nc()
    res = run_bass_kernel_spmd(nc, in_maps, core_ids=list(range(8)))
    out = np.zeros((2, DIM, N), np.float32)
    for core in range(8):
        b = core // 4
        part = np.asarray(res.results[core]["out"], dtype=np.float32)
        out[b, 0:128, :] += part[0]
        out[b, 128:256, :] += part[1]
    return out.reshape(2, DIM, 48, 48)


# revision 32
# speedup vs baseline: 1.0299x; 1.0299x over previous
"""Trainium2 Bass kernel for nn_Attention (dense transformer attention block).

Reference computation (shapes fixed):
  x [2, 256, 48, 48] -> RMSNorm over channels -> 1x1 conv to qkv (8 heads, 64 dhead)
  -> prepend 4 learnable mem kv tokens -> softmax attention -> 1x1 conv out [2, 256, 48, 48]

Sharding (v4, head-sharded, collective-free): 8 cores = 2 batches x 4
head-pairs. Core c handles batch c//4 and heads (2g, 2g+1), g = c%4. Each core
runs its heads' attention over ALL 2304 queries (512-wide chunks keep the PE
at peak column rate) and emits its PARTIAL out-projection [256, 2304] in f32,
DMA'd straight from PSUM. The host sums the 4 partials per batch -- no
on-device collective at all.

Key structure:
  - x is L2-normalized ONCE (xn = x * sigma, sigma=16/l2 per position, bf16)
    before the qkv projections, so q, k, v all come out normalized: no per-key
    exp bias, no sigma column machinery; the attnv denominator column is
    exactly 1.0.
  - attnv lhsT staging per key tile: [keys, 2 heads, 128], col 0 = 1.0 (den),
    cols 64:128 = v^T; acc row 0 = denominator, rows 64:128 = attention out.
  - exp split across ACT (table exp) and DVE (Schraudolph bf16 bit-trick).
  - PE pipeline: sim(jt+1) emitted before attnv(jt); sim head pairs run
    concurrently on disjoint PE row groups (K=64 at base partitions 0/64).
  - engine placement: all input DMAs on the sync queue; gpsimd = xn muls +
    staging zero-memsets + partition broadcasts; ACT = squares/sqrts/qb cast +
    exp share; DVE = kb/v staging, oT normalize, recips + exp share.
"""
import math

import numpy as np

import concourse.mybir as mybir
import concourse.tile as tile
from concourse import bacc
from concourse.bass_utils import run_bass_kernel_spmd


F32 = mybir.dt.float32
F32R = mybir.dt.float32r
BF16 = mybir.dt.bfloat16
U16 = mybir.dt.uint16
EXP = mybir.ActivationFunctionType.Exp
SQRT = mybir.ActivationFunctionType.Sqrt
SQUARE = mybir.ActivationFunctionType.Square
COPY = mybir.ActivationFunctionType.Copy
MULT = mybir.AluOpType.mult
ADD = mybir.AluOpType.add

DIM = 256
HEADS = 8
DHEAD = 64
MEM = 4
HID = 512
N = 48 * 48          # 2304 image positions
NJT = 18             # image key tiles of 128

CHUNKS = [(0, 512), (512, 512), (1024, 512), (1536, 512), (2048, 256)]
NCH = len(CHUNKS)
DVE_JTS = frozenset({2, 4, 7, 9, 11, 14, 16, 17})  # interleaved with ACT

A_SCH = 128.0 / math.log(2.0)        # bf16 Schraudolph slope
B_SCH = 127.0 * 128.0 - 5.5          # minimax-centered magic constant


def _jt_slice(jt):
    """key tile jt -> (chunk index, col offset within chunk)."""
    pos0 = jt * 128
    for ci, (c0, cw) in enumerate(CHUNKS):
        if c0 <= pos0 < c0 + cw:
            return ci, pos0 - c0
    raise AssertionError(jt)


def build():
    nc = bacc.Bacc("TRN2", target_bir_lowering=False, debug=False,
                   enable_asserts=True, num_devices=8)
    x_d = nc.dram_tensor("x", [DIM, N], F32, kind="ExternalInput").ap()
    wqkv_d = nc.dram_tensor("wqkv", [DIM, 384], F32, kind="ExternalInput").ap()
    memk_d = nc.dram_tensor("memk", [128, MEM], F32, kind="ExternalInput").ap()
    memvst_d = nc.dram_tensor("memvst", [MEM, 2, 128], F32,
                              kind="ExternalInput").ap()
    woutT_d = nc.dram_tensor("woutT", [2, DHEAD, DIM], F32,
                             kind="ExternalInput").ap()
    out_d = nc.dram_tensor("out", [2, 128, N], BF16,
                           kind="ExternalOutput").ap()

    with tile.TileContext(nc) as tc:
        with (
            tc.tile_pool(name="consts", bufs=1) as consts,
            tc.tile_pool(name="big", bufs=1) as big,
            tc.tile_pool(name="io", bufs=1) as io,
            tc.tile_pool(name="wk", bufs=2) as wk,
            tc.tile_pool(name="ps_s", bufs=2, space="PSUM") as ps_s,
            tc.tile_pool(name="ps_a", bufs=2, space="PSUM") as ps_a,
        ):
            # ---------------- input DMAs (sync queue, priority order) ------
            xs = [[None] * NCH, [None] * NCH]
            for ci, (c0, cw) in enumerate(CHUNKS):
                for kt in range(2):
                    xs[kt][ci] = big.tile([128, 512], F32, tag=f"x{kt}_{ci}",
                                          name=f"x{kt}_{ci}")
            wq_f = io.tile([128, 2, 384], F32, tag="wq_f")
            memk_f = io.tile([128, MEM], F32, tag="memk_f")
            memvst_f = io.tile([MEM, 2, 128], F32, tag="memv_f")
            woutA_f = io.tile([128, DIM], F32, tag="woutA_f")
            woutB_f = io.tile([128, DIM], F32, tag="woutB_f")

            nc.sync.dma_start(out=xs[0][0][:, :], in_=x_d[0:128, 0:512])
            nc.gpsimd.dma_start(out=xs[1][0][:, :], in_=x_d[128:256, 0:512])
            nc.sync.dma_start(out=wq_f[:, 0, :], in_=wqkv_d[0:128, :])
            nc.gpsimd.dma_start(out=wq_f[:, 1, :], in_=wqkv_d[128:256, :])
            for ci, (c0, cw) in enumerate(CHUNKS[1:], start=1):
                nc.sync.dma_start(out=xs[0][ci][:, 0:cw],
                                  in_=x_d[0:128, c0:c0 + cw])
                nc.sync.dma_start(out=xs[1][ci][:, 0:cw],
                                  in_=x_d[128:256, c0:c0 + cw])
            nc.sync.dma_start(out=memk_f[:, :], in_=memk_d)
            nc.sync.dma_start(out=memvst_f[:, :, :], in_=memvst_d)
            nc.sync.dma_start(out=woutA_f[64:128, :], in_=woutT_d[0, :, :])
            nc.gpsimd.dma_start(out=woutB_f[64:128, :], in_=woutT_d[1, :, :])

            # ---------------- constants / staging init --------------------
            ones_f = consts.tile([128, 1], F32)
            nc.vector.memset(ones_f[:, :], 1.0)
            ones_r = consts.tile([128, 128], F32R)
            nc.vector.tensor_copy(ones_r[:, :],
                                  ones_f[:, :].to_broadcast((128, 128)))



            # attnv lhsT staging: [keys, 2 heads, 128]
            # col 0 = 1.0 (denominator), 1:64 zeros, 64:128 = v^T
            vstag = [big.tile([128, 2, 128], BF16, tag=f"vst{jt}",
                              name=f"vst{jt}")
                     for jt in range(NJT + 1)]
            for jt in range(NJT):
                nc.vector.memset(vstag[jt][:, :, 0:1], 1.0)

            # weight conversion: bf16 qkv weights
            wq = consts.tile([128, 2, 384], BF16, tag="wq")
            nc.vector.tensor_copy(wq[:, :, :], wq_f[:, :, :])

            kb = [None] * NCH
            qb = [None] * NCH

            # ---------------- prep: per-chunk qkv --------------------------
            def prep_chunk(ci):
                c0, cw = CHUNKS[ci]
                njs = cw // 128
                xsq = wk.tile([128, 2, 512], F32R, tag="xsq", name=f"xsq{ci}")
                nc.scalar.activation(xsq[:, 0, 0:cw],
                                     xs[0][ci][:, 0:cw], SQUARE)
                nc.vector.tensor_mul(xsq[:, 1, 0:cw], xs[1][ci][:, 0:cw],
                                     xs[1][ci][:, 0:cw])
                ssq = ps_a.tile([128, 2, 512], F32, tag="a", name=f"ssq{ci}")
                for kt in range(2):
                    nc.tensor.matmul(ssq[:, 0, 0:cw], ones_r[:, :],
                                     xsq[:, kt, 0:cw],
                                     start=(kt == 0), stop=(kt == 1))
                # sigma rows = 16/l2 (sqrt then fast reciprocal)
                srow = wk.tile([128, 512], F32, tag="sr", name=f"sr{ci}")
                nc.scalar.activation(srow[:, 0:cw], ssq[:, 0, 0:cw], SQRT,
                                     scale=1.0 / 256.0)
                nc.vector.reciprocal_approx_fast(srow[:, 0:cw], srow[:, 0:cw])
                # xn = x * sigma (normalized x, bf16 for the matmuls)
                xn = wk.tile([128, 2, 512], BF16, tag="xn", name=f"xn{ci}")
                nc.vector.tensor_mul(xn[:, 0, 0:cw], xs[0][ci][:, 0:cw],
                                     srow[:, 0:cw])
                nc.gpsimd.tensor_mul(xn[:, 1, 0:cw], xs[1][ci][:, 0:cw],
                                     srow[:, 0:cw])
                # q, k projections
                qk = ps_s.tile([128, 2, 512], F32, tag="s", name=f"qk_{ci}")
                for m in range(2):  # 0 = q, 1 = k
                    for kt in range(2):
                        nc.tensor.matmul(
                            qk[:, m, 0:cw],
                            wq[:, kt, m * 128:m * 128 + 128],
                            xn[:, kt, 0:cw],
                            start=(kt == 0), stop=(kt == 1))
                qb[ci] = big.tile([128, 512], BF16, tag=f"q{ci}",
                                  name=f"qb{ci}")
                kb[ci] = big.tile([128, 512], BF16, tag=f"k{ci}",
                                  name=f"kb{ci}")
                nc.scalar.activation(qb[ci][:, 0:cw], qk[:, 0, 0:cw], COPY)
                nc.vector.tensor_copy(kb[ci][:, 0:cw], qk[:, 1, 0:cw])
                # v, directly transposed: [pos, 2 heads, 64]
                vps = ps_a.tile([128, 2, 4, 2, 64], F32, tag="a",
                                name=f"vps{ci}")
                for js in range(njs):
                    jt = ci * 4 + js
                    nc.gpsimd.memset(vstag[jt][:, :, 1:64], 0.0)
                    for kt in range(2):
                        nc.tensor.matmul(
                            vps[:, 0, js, :, :],
                            xn[:, kt, js * 128:js * 128 + 128],
                            wq[:, kt, 256:384],
                            start=(kt == 0), stop=(kt == 1))
                    nc.vector.tensor_copy(vstag[jt][:, :, 64:128],
                                          vps[:, 0, js, :, :])

            for ci in range(NCH):
                prep_chunk(ci)
            # late consts (off the critical path at start)
            kmem = consts.tile([128, MEM], BF16, tag="kmem")
            nc.vector.tensor_copy(kmem[:, :], memk_f[:, :])
            nc.vector.tensor_copy(vstag[NJT][0:MEM, :, :], memvst_f[:, :, :])
            woutA = consts.tile([128, DIM], BF16, tag="woutA")
            woutB = consts.tile([128, DIM], BF16, tag="woutB")
            nc.vector.tensor_copy(woutA[64:128, :], woutA_f[64:128, :])
            nc.vector.tensor_copy(woutB[64:128, :], woutB_f[64:128, :])
            wouts = [woutA, woutB]

            # ---------------- attention chunks ------------------------------
            accs_by_ci = [None] * NCH
            fin = {}

            def emit_sim(ci, jt):
                c0, cw = CHUNKS[ci]
                s_ps = ps_s.tile([128, 2, 512], F32, tag="s",
                                 name=f"s_{ci}_{jt}")
                if jt < NJT:
                    km = 128
                    kc, off = _jt_slice(jt)
                    klhs = [kb[kc][64 * h:64 * h + 64, off:off + 128]
                            for h in range(2)]
                else:
                    km = MEM
                    klhs = [kmem[64 * h:64 * h + 64, :] for h in range(2)]
                for h in range(2):
                    nc.tensor.matmul(
                        s_ps[0:km, h, 0:cw],
                        klhs[h],
                        qb[ci][64 * h:64 * h + 64, 0:cw],
                        start=True, stop=True)
                return s_ps, km

            def emit_exp(ci, jt, s_ps, km):
                c0, cw = CHUNKS[ci]
                if jt in DVE_JTS:
                    P = wk.tile([128, 2, 512], U16, tag="Pd",
                                name=f"Pd_{ci}_{jt}")
                    nc.vector.tensor_scalar(
                        out=P[:, :, 0:cw], in0=s_ps[:, :, 0:cw],
                        scalar1=A_SCH, scalar2=B_SCH,
                        op0=MULT, op1=ADD)
                    return P, True
                P = wk.tile([128, 2, 512], BF16, tag="Pa", name=f"Pa_{ci}_{jt}")
                nc.scalar.activation(P[0:km, :, 0:cw], s_ps[0:km, :, 0:cw],
                                     EXP)
                return P, False

            def emit_attnv(ci, jt, P, km, cast):
                c0, cw = CHUNKS[ci]
                acc = accs_by_ci[ci]
                for h in range(2):
                    rhs = P[0:km, h, 0:cw]
                    if cast:
                        rhs = rhs.bitcast(BF16)
                    nc.tensor.matmul(
                        acc[:, h, 0:cw],
                        vstag[jt][0:km, h, :],
                        rhs,
                        start=(jt == 0), stop=(jt == NJT),
                        skip_group_check=True)

            def emit_fin_a(ci):
                """den reciprocal + partition broadcast."""
                c0, cw = CHUNKS[ci]
                acc = accs_by_ci[ci]
                rec = wk.tile([128, 2, 512], F32, tag="rec", name=f"rec{ci}")
                rb = wk.tile([128, 2, 512], F32, tag="rb", name=f"rb{ci}")
                nc.vector.reciprocal_approx_fast(rec[0:1, :, 0:cw],
                                                 acc[0:1, :, 0:cw])
                nc.gpsimd.partition_broadcast(rb[:, :, 0:cw],
                                              rec[0:1, :, 0:cw])
                fin[ci] = rb

            def emit_fin_b(ci):
                """normalize oT (DVE)."""
                c0, cw = CHUNKS[ci]
                acc = accs_by_ci[ci]
                rb = fin.pop(ci)
                oT = wk.tile([128, 2, 512], BF16, tag="oT", name=f"oT_{ci}")
                nc.vector.tensor_mul(oT[64:128, :, 0:cw],
                                     acc[64:128, :, 0:cw],
                                     rb[64:128, :, 0:cw])
                fin[ci] = oT

            def emit_fin_c(ci):
                """partial out-projection (PE)."""
                c0, cw = CHUNKS[ci]
                oT = fin.pop(ci)
                op = ps_s.tile([128, 2, 512], F32, tag="s", name=f"op_{ci}")
                for mt in range(2):
                    for h in range(2):
                        nc.tensor.matmul(
                            op[:, mt, 0:cw],
                            wouts[h][64:128, mt * 128:mt * 128 + 128],
                            oT[64:128, h, 0:cw],
                            start=(h == 0), stop=(h == 1))
                fin[ci] = op

            def emit_fin_d(ci):
                """osb cast (ACT) + output DMA (sync)."""
                c0, cw = CHUNKS[ci]
                op = fin.pop(ci)
                osb = wk.tile([128, 2, 512], BF16, tag="osb", name=f"osb{ci}")
                nc.scalar.activation(osb[:, :, 0:cw], op[:, :, 0:cw], COPY)
                for mt in range(2):
                    nc.sync.dma_start(out=out_d[mt, :, c0:c0 + cw],
                                      in_=osb[:, mt, 0:cw])

            def hooks(ci, jt):
                if ci == 0:
                    return
                if jt == 3:
                    emit_fin_a(ci - 1)
                elif jt == 5:
                    emit_fin_b(ci - 1)
                elif jt == 8:
                    emit_fin_c(ci - 1)
                elif jt == 10:
                    emit_fin_d(ci - 1)

            for ci, (c0, cw) in enumerate(CHUNKS):
                accs_by_ci[ci] = ps_a.tile([128, 2, 512], F32, tag="a",
                                           name=f"acc_{ci}")
                pend = None
                for jt in range(NJT + 1):
                    s_ps, km = emit_sim(ci, jt)
                    if pend is not None:
                        emit_attnv(ci, *pend)
                    hooks(ci, jt)
                    P, cast = emit_exp(ci, jt, s_ps, km)
                    pend = (jt, P, km, cast)
                emit_attnv(ci, *pend)
            emit_fin_a(NCH - 1)
            emit_fin_b(NCH - 1)
            emit_fin_c(NCH - 1)
            emit_fin_d(NCH - 1)
    nc.compile()
    return nc


_NC = None
_last_in_maps = None


def _get_nc():
    global _NC
    if _NC is None:
        _NC = build()
    return _NC


def make_in_maps(x, gamma, mem_kv, w_qkv, w_out):
    x = np.asarray(x, np.float32)
    gamma = np.asarray(gamma, np.float32).reshape(DIM)
    mem_kv = np.asarray(mem_kv, np.float32)
    w_qkv = np.asarray(w_qkv, np.float32)
    w_out = np.asarray(w_out, np.float32)

    g1 = 1.0 + gamma  # [256]
    scale = DHEAD ** -0.5
    in_maps = []
    for core in range(8):
        b, g = core // 4, core % 4
        hA, hB = 2 * g, 2 * g + 1
        blocks = []
        for t in range(3):  # q, k, v
            for h in (hA, hB):
                wblk = w_qkv[t * HID + h * DHEAD: t * HID + (h + 1) * DHEAD, :]
                if t == 0:
                    wblk = wblk * scale
                blocks.append(wblk.T)  # [256, 64]
        wqkvT = np.concatenate(blocks, axis=1) * g1[:, None]  # [256, 384]
        memk = np.concatenate(
            [mem_kv[0, hA].T, mem_kv[0, hB].T], axis=0)  # [128, 4]
        # mem staging: [4, 2, 128] = [1.0 (den) | zeros | v (64:128)]
        memvst = np.zeros((MEM, 2, 128), np.float32)
        memvst[:, :, 0] = 1.0
        memvst[:, 0, 64:128] = mem_kv[1, hA]
        memvst[:, 1, 64:128] = mem_kv[1, hB]
        # wout rows: head A weights at partitions 64:128, head B at 0:64
        woutT = np.stack(
            [w_out[:, hA * DHEAD:(hA + 1) * DHEAD].T,
             w_out[:, hB * DHEAD:(hB + 1) * DHEAD].T], axis=0)  # [2, 64, 256]
        in_maps.append({
            "x": np.ascontiguousarray(x[b].reshape(DIM, N)),
            "wqkv": np.ascontiguousarray(wqkvT),
            "memk": np.ascontiguousarray(memk),
            "memvst": np.ascontiguousarray(memvst),
            "woutT": np.ascontiguousarray(woutT),
        })
    return in_maps


def kernel(x, gamma, mem_kv, w_qkv, w_out):
    global _last_in_maps
    in_maps = make_in_maps(x, gamma, mem_kv, w_qkv, w_out)
    _last_in_maps = in_maps
    nc = _get_nc()
    res = run_bass_kernel_spmd(nc, in_maps, core_ids=list(range(8)))
    out = np.zeros((2, DIM, N), np.float32)
    for core in range(8):
        b = core // 4
        part = np.asarray(res.results[core]["out"], dtype=np.float32)
        out[b, 0:128, :] += part[0]
        out[b, 128:256, :] += part[1]
    return out.reshape(2, DIM, 48, 48)
